# revision 47
# baseline (speedup 1.0000x reference)
"""Trainium2 Bass kernel for 4-layer cross-stencil CNN.

Per-core: one image [6,256,256] (batch dim sharded across 8 cores).
conv(cross-5-stencil) = 5 channel-matmuls with spatially shifted rhs APs,
accumulated in PSUM. Channels on partitions, spatial (rows x cols) on the
free dim. fp32r matmuls (full PE rate at N>=256).

Rolling-strip pipeline: strip k produces R fresh rows of every layer
(L1 leads by 3 rows, L2 by 2, L3/t5 by 1), so each row of each layer is
computed exactly once -- no halo recompute. The 2-row inter-strip halos
of h1/h2/t5 are carried by tiny shift-copies. x is host-packed into 5
pre-shifted zero-padded tap groups (one DMA descriptor per strip, no
pad memsets) and double-buffered; at the strip seam the next strip's
first L1 chunks are pre-rolled so the PE has ready work while the last
mmA h3-copies land.

L1 and L2 are emitted interleaved (an L1 chunk unlocks the L2 chunk
~5 chunks behind it): a separate L1 burst is PSUM-pool-recycle bound,
since the PE retires K=31 L1 chunks at 213ns while their relu copies
drain at ~630ns on two engines.  The startup pad memsets and the t5
inter-strip halo copy run on the otherwise-idle Pool (GPSIMD) engine so
they never head-of-line-block the DVE/ACT queues (GPSIMD cannot touch
PSUM, so the relu copies themselves must stay on DVE/ACT).  The first
strip's x load is split in two so early L1 chunks gate on a smaller
transfer.

L1 packs the 5 taps into K=30 plus a host-packed all-ones 31st
channel whose weight row is b1, so the PSUM result already carries the
bias and the relu copies have no bias-load dependency. L4 computes
the 5 taps as one M=128 matmul whose output slabs sit at partitions
0/32/64/96 (legal engine bases) plus center at 6; a DMA slab-gather
pre-shifts them into t5s and one K=128 bf16 selector matmul sums them.
The tap-sum for strip k-1 is emitted interleaved into strip k's L2
phase; output rows stage in SBUF so y needs one DMA per half-strip.
The final strip instead chases the mmA drain with low-latency DVE
slab-copies (4-row pieces, 2+2 at the end so the penultimate piece
clears one mmA chunk early) and takes the center tap directly from h3
(second accumulating matmul), keeping the drain tail ~1 small piece
deep. Engine assignment (DVE/ACT alternation per consumer) is tuned
against the TimelineSim cost model: HWDGE descriptors are ~630ns
serialized with ~900ns completion-semaphore latency, so descriptor
count is minimized and every DMA->compute dependency is issued well
ahead of its consumer.
"""

import sys

sys.path.insert(0, "/opt/trn_rl_repo")

import ml_dtypes
import numpy as np

import concourse.bacc as bacc
import concourse.mybir as mybir
from concourse.tile import TileContext
from concourse import bass_utils

IN_C, HID_C, OUT_C = 6, 128, 6
B, H, W = 8, 256, 256
WP = W + 2  # padded width
R = 24  # output rows per strip
N_CORES = 8

f32 = mybir.dt.float32
f32r = mybir.dt.float32r
bf16 = mybir.dt.bfloat16
Add = mybir.AluOpType.add
Max = mybir.AluOpType.max
Relu = mybir.ActivationFunctionType.Relu
Ident = mybir.ActivationFunctionType.Identity

# tap order matches reference: 0=center, 1=up(x[h-1]), 2=down(x[h+1]),
# 3=left(x[w-1]), 4=right(x[w+1])


def _build(repeat=1):
    nc = bacc.Bacc("TRN2", target_bir_lowering=False)

    # x5p: host-packed 5 pre-shifted copies of x (tap order, zero-padded),
    # so one DMA per strip fills all 5 tap groups and no pads are needed
    x_d = nc.dram_tensor("x5p", [5 * IN_C + 1, H, W], f32, kind="ExternalInput")
    # xw0: w1p (padded to a row) + x rows 0-3, one startup descriptor
    xw0_d = nc.dram_tensor("xw0", [5 * IN_C + 1, 5, W], f32, kind="ExternalInput")
    w2_d = nc.dram_tensor("w2p", [HID_C, 5, HID_C], f32, kind="ExternalInput")
    w3_d = nc.dram_tensor("w3p", [HID_C, 5, HID_C], f32, kind="ExternalInput")
    # w4a: all 5 taps as M=128 slabs: up@0-5, center@6-11, down@32-37,
    # left@64-69, right@96-101; zero elsewhere
    w4a_d = nc.dram_tensor("w4a", [HID_C, HID_C], f32, kind="ExternalInput")
    # s6: bf16 selector summing the 5 (pre-shifted) slabs of t5s
    s6_d = nc.dram_tensor("s6", [HID_C, OUT_C], bf16, kind="ExternalInput")
    # s6b: s6 without the center block; w4c: center-tap weights (for the
    # last strip the center contribution is a direct matmul from h3)
    s6b_d = nc.dram_tensor("s6b", [HID_C, OUT_C], bf16, kind="ExternalInput")
    w4c_d = nc.dram_tensor("w4c", [HID_C, OUT_C], f32, kind="ExternalInput")
    b1_d = nc.dram_tensor("b1", [HID_C], f32, kind="ExternalInput")
    b2_d = nc.dram_tensor("b2", [HID_C], f32, kind="ExternalInput")
    b3_d = nc.dram_tensor("b3", [HID_C], f32, kind="ExternalInput")
    b4_d = nc.dram_tensor("b4", [OUT_C], f32, kind="ExternalInput")
    y_d = nc.dram_tensor("y", [OUT_C, H, W], f32, kind="ExternalOutput")

    with TileContext(nc) as tc:
        with (
            tc.tile_pool(name="const", bufs=1) as cpool,
            tc.tile_pool(name="bufs", bufs=1) as bpool,
            tc.tile_pool(name="io", bufs=4) as iopool,
            tc.tile_pool(name="psmain", bufs=8, space="PSUM") as pmain,
        ):
            # --- persistent strip buffers ---
            # x30: 5 tap-groups x 6ch, all pre-shifted on host; double
            # buffered per strip. L1 row r reads slot r-a+1 in every group.
            xbufs = [
                bpool.tile([5 * IN_C + 1, R + 4, WP], f32r, name=f"x30_{i}")
                for i in range(2)
            ]
            h1 = bpool.tile([HID_C, R + 4, WP], f32r)  # row r at slot r-a+1
            h2 = bpool.tile([HID_C, R + 3, WP], f32r)  # row r at slot r-a+1
            h3 = bpool.tile([HID_C, R + 1, WP], f32r)  # row r at slot r-a
            # t5: tap partials (slabs up@0,cen@6,dn@32,lf@64,rt@96), bf16,
            # row r at slot r-a+1
            t5 = bpool.tile([HID_C, R + 2, WP], bf16)
            # t5s: DMA-gathered pre-shifted taps; slot d = output row a+d
            t5s = bpool.tile([HID_C, R, WP], bf16)
            # per-half-strip output staging (one y DMA per half-strip)
            ybufs = [
                bpool.tile([OUT_C, R // 2, W], f32, name=f"ybuf_{i}")
                for i in range(2)
            ]

            def load_x(k, a, p1, q1):
                """One descriptor: all 5 pre-shifted groups for L1 rows
                [p1, q1) land at slots [p1-a+1, q1-a+1)."""
                nc.sync.dma_start(
                    out=xbufs[k % 2][:, p1 - a + 1 : q1 - a + 1, 1 : 1 + W],
                    in_=x_d[:, p1:q1, :].bitcast(f32r),
                )

            # critical-path startup: ONE descriptor carries w1 (into the
            # never-used slot 0 of x buffer 0; interior reloads only touch
            # slots >= 4, so it stays resident) plus x rows 0-3
            nc.sync.dma_start(
                out=xbufs[0][:, 0:5, 1 : 1 + W],
                in_=xw0_d[:, :, :].bitcast(f32r))
            w1_sb = xbufs[0][:, 0, 1 : 1 + HID_C]
            load_x(0, 0, 4, 14)
            load_x(0, 0, 14, R + 3)

            # --- remaining weights / biases (resident) ---
            w2_sb = cpool.tile([HID_C, 5, HID_C], f32r)
            nc.sync.dma_start(out=w2_sb, in_=w2_d[:, :, :].bitcast(f32r))
            w3_sb = cpool.tile([HID_C, 5, HID_C], f32r)
            nc.sync.dma_start(out=w3_sb, in_=w3_d[:, :, :].bitcast(f32r))
            w4a_sb = cpool.tile([HID_C, HID_C], f32r)
            nc.sync.dma_start(out=w4a_sb, in_=w4a_d[:, :].bitcast(f32r))
            s6_sb = cpool.tile([HID_C, OUT_C], bf16)
            nc.sync.dma_start(out=s6_sb, in_=s6_d[:, :])
            s6b_sb = cpool.tile([HID_C, OUT_C], bf16)
            nc.sync.dma_start(out=s6b_sb, in_=s6b_d[:, :])
            w4c_sb = cpool.tile([HID_C, OUT_C], f32r)
            nc.sync.dma_start(out=w4c_sb, in_=w4c_d[:, :].bitcast(f32r))
            b2_sb = cpool.tile([HID_C, 1], f32)
            nc.sync.dma_start(out=b2_sb, in_=b2_d[:, None])
            b3_sb = cpool.tile([HID_C, 1], f32)
            nc.sync.dma_start(out=b3_sb, in_=b3_d[:, None])
            b4_sb = cpool.tile([OUT_C, 1], f32)
            nc.sync.dma_start(out=b4_sb, in_=b4_d[:, None])

            # remaining pads
            nc.gpsimd.memset(h1[:, :, 0:1].bitcast(f32), 0.0)
            nc.gpsimd.memset(h1[:, :, 257:258].bitcast(f32), 0.0)
            nc.gpsimd.memset(h1[:, 0:1, :].bitcast(f32), 0.0)
            nc.gpsimd.memset(h2[:, :, 0:1].bitcast(f32), 0.0)
            nc.gpsimd.memset(h2[:, :, 257:258].bitcast(f32), 0.0)
            nc.gpsimd.memset(h2[:, 0:1, :].bitcast(f32), 0.0)
            nc.gpsimd.memset(t5[:, :, 0:1], 0.0)
            nc.gpsimd.memset(t5[:, :, 257:258], 0.0)
            nc.gpsimd.memset(t5[:, 0:1, :], 0.0)
            # t5s garbage partitions are never gathered; selector rows are
            # zero there but 0*garbage must not be NaN -> zero once
            nc.gpsimd.memset(t5s[:, :, :], 0.0)

            def conv_chunk(ps, w_sb, src, s, n):
                """5 accumulating matmuls; center is src[:, s:s+n, 1:1+W]."""
                nc.tensor.matmul(
                    ps, w_sb[:, 0, :], src[:, s : s + n, 1 : 1 + W],
                    start=True, stop=False,
                )
                nc.tensor.matmul(
                    ps, w_sb[:, 1, :], src[:, s - 1 : s - 1 + n, 1 : 1 + W],
                    start=False, stop=False,
                )
                nc.tensor.matmul(
                    ps, w_sb[:, 2, :], src[:, s + 1 : s + 1 + n, 1 : 1 + W],
                    start=False, stop=False,
                )
                nc.tensor.matmul(
                    ps, w_sb[:, 3, :], src[:, s : s + n, 0:W],
                    start=False, stop=False,
                )
                nc.tensor.matmul(
                    ps, w_sb[:, 4, :], src[:, s : s + n, 2 : 2 + W],
                    start=False, stop=True,
                )

            def gather_piece(a, h0, h1r, fast=False, center=True, pool=False):
                """Pre-shifted slab gather of t5 -> t5s for out rows
                [a+h0, a+h1r) via flat SBUF->SBUF DMA. fast=True moves the
                quadrant-aligned slabs onto DVE/ACT (lower latency than the
                serialized DMA queue); only the center slab (partition base
                6) must stay a DMA."""
                t5f = t5.rearrange("p r c -> p (r c)")
                t5sf = t5s.rearrange("p r c -> p (r c)")
                o0, o1 = h0 * WP, h1r * WP
                if center:
                    nc.scalar.dma_start(
                        out=t5sf[6:12, o0:o1],
                        in_=t5f[6:12, WP + o0 : WP + o1])
                if fast:
                    cp = nc.gpsimd.tensor_copy if pool else nc.vector.tensor_copy
                    cp(t5s[0:6, h0:h1r, :], t5[0:6, h0:h1r, :])
                    cp(t5s[32:38, h0:h1r, :], t5[32:38, h0 + 2 : h1r + 2, :])
                    cp(t5sf[64:70, o0 + 1 : o1],
                       t5f[64:70, WP + o0 : WP + o1 - 1])
                    cp(t5sf[96:102, o0 : o1 - 1],
                       t5f[96:102, WP + o0 + 1 : WP + o1])
                    return
                nc.scalar.dma_start(out=t5sf[0:6, o0:o1], in_=t5f[0:6, o0:o1])
                nc.scalar.dma_start(
                    out=t5sf[32:38, o0:o1],
                    in_=t5f[32:38, 2 * WP + o0 : 2 * WP + o1])
                nc.scalar.dma_start(
                    out=t5sf[64:70, o0 + 1 : o1],
                    in_=t5f[64:70, WP + o0 : WP + o1 - 1])
                nc.scalar.dma_start(
                    out=t5sf[96:102, o0 : o1 - 1],
                    in_=t5f[96:102, WP + o0 + 1 : WP + o1])

            def l4b_chunks(a, r0, r1, yb, yb0, direct=False):
                """Deferred emitters for out rows [r0, r1): per 2-row chunk
                one K=128 selector matmul sums the 5 pre-shifted slabs of
                t5s; bias lands in ybufs[yb] at slot rr-yb0. direct=True
                (last strip, h3 still valid) takes the center tap straight
                from h3 so t5s center needs no gather."""
                out = []
                rr = r0
                while rr < r1:
                    n = min(2, r1 - rr)

                    def emit(rr=rr, n=n, a=a, yb0=yb0, yb=yb,
                             direct=direct and rr != a):
                        d = rr - a
                        ps = pmain.tile([OUT_C, n, W], f32, tag="ps")
                        if direct:
                            nc.tensor.matmul(
                                ps, s6b_sb[:, :], t5s[:, d : d + n, 1 : 1 + W],
                                start=True, stop=False,
                            )
                            nc.tensor.matmul(
                                ps, w4c_sb[:, :], h3[:, d : d + n, 1 : 1 + W],
                                start=False, stop=True,
                            )
                        else:
                            nc.tensor.matmul(
                                ps, s6_sb[:, :], t5s[:, d : d + n, 1 : 1 + W],
                                start=True, stop=True,
                            )
                        if ((d // 2) % 2 == 0 and not direct) or (
                                direct and d >= 14):
                            nc.vector.tensor_scalar_add(
                                ybufs[yb][:, rr - yb0 : rr - yb0 + n, :], ps,
                                b4_sb)
                        else:
                            nc.scalar.activation(
                                ybufs[yb][:, rr - yb0 : rr - yb0 + n, :], ps,
                                Ident, bias=b4_sb)

                    out.append(emit)
                    rr += n
                return out

            def y_flush(r0, r1, yb):
                nc.scalar.dma_start(
                    out=y_d[:, r0:r1, :], in_=ybufs[yb][:, 0 : r1 - r0, :])

            def l4b_half(a, r0, r1, yb, direct=False):
                return l4b_chunks(a, r0, r1, yb, r0, direct=direct) + [
                    lambda: y_flush(r0, r1, yb)
                ]

            pending = []  # tap-sum emitters from the previous strip
            PRE = 2  # L1 chunks pre-rolled across the strip seam
            starts = list(range(0, H, R))
            for rep in range(repeat):
             for k, a in enumerate(starts):
                b = starts[k + 1] if k + 1 < len(starts) else H
                first = a == 0
                last = b == H
                p1, q1 = (0 if first else a + 3), min(H, b + 3)
                p2, q2 = (0 if first else a + 2), min(H, b + 2)
                p3, q3 = (0 if first else a + 1), min(H, b + 1)

                if last:
                    # zero the single slot holding row H (stale from a
                    # previous strip); t5's is deferred below (its old slot
                    # content is still read by strip k-1's deferred gather)
                    nc.vector.memset(
                        h1[:, q1 - a + 1 : q1 - a + 2, :].bitcast(f32), 0.0)
                    nc.vector.memset(
                        h2[:, q2 - a + 1 : q2 - a + 2, :].bitcast(f32), 0.0)

                np_ = a - starts[k - 1] if k > 0 else 0  # prev strip size
                if not first:
                    # carry the 2-row inter-strip halos of h1/h2 (rows
                    # [a+1,a+3) / [a,a+2)) from the previous strip's frame;
                    # must precede this strip's L1/L2 writes of the source
                    # slots (WAR by program order); on ACT: the DVE queue
                    # is the congested one here
                    nc.scalar.copy(
                        h1[:, 2:4, :], h1[:, np_ + 2 : np_ + 4, :])
                    nc.scalar.copy(
                        h2[:, 1:3, :], h2[:, np_ + 1 : np_ + 3, :])

                def l1_chunk(rr, n, ci, a=a, xb=xbufs[k % 2]):
                    s = rr - a + 1
                    ps = pmain.tile([HID_C, n, W], f32, tag="ps")
                    nc.tensor.matmul(
                        ps, w1_sb, xb[:, s : s + n, 1 : 1 + W],
                        start=True, stop=True,
                    )
                    if ci % 2 == 0:
                        nc.vector.tensor_scalar(
                            h1[:, s : s + n, 1 : 1 + W], ps, 0.0, 0.0, Add, Max
                        )
                    else:
                        nc.scalar.activation(
                            h1[:, s : s + n, 1 : 1 + W], ps, Relu
                        )

                # t5 halo carry (rows [a-1,a+1)): on Pool, so it no longer
                # head-of-line-blocks the DVE queue; precedes this strip's
                # mmA overwrites and the gather read
                if not first:
                    nc.gpsimd.tensor_copy(t5[:, 0:2, :], t5[:, np_ : np_ + 2, :])

                # next strip's x load: queued here so it lands well before
                # its L1 phase (WAR on strip k-1's reads is already clear)
                if not last:
                    b_n = min(b + R, H)
                    load_x(k + 1, b, b + 3, min(H, b_n + 3))

                # --- L1 + L2 interleaved: each L1 chunk unlocks the L2
                # chunk ~2 behind it; a separate L1 burst is PSUM-pool
                # recycle bound (PE emits 213ns chunks against ~630ns
                # copies), interleaving keeps the PE fed ---
                l2ci = [0]

                def emit_l2(rr2, n2):
                    s = rr2 - a + 1  # h1 slot of center
                    ps = pmain.tile([HID_C, n2, W], f32, tag="ps")
                    conv_chunk(ps, w2_sb, h1, s, n2)
                    if l2ci[0] % 3 == 2:
                        nc.vector.tensor_scalar(
                            h2[:, s : s + n2, 1 : 1 + W], ps, b2_sb,
                            0.0, Add, Max,
                        )
                    else:
                        nc.scalar.activation(
                            h2[:, s : s + n2, 1 : 1 + W], ps, Relu, bias=b2_sb
                        )
                    l2ci[0] += 1
                    if pending:
                        pending.pop(0)()

                rr = p1 + (0 if first else 2 * PRE)
                ci = 0 if first else PRE
                rr2 = p2
                while rr < q1:
                    n = min(2, q1 - rr)
                    l1_chunk(rr, n, ci)
                    ci += 1
                    rr += n
                    while rr2 < q2 and rr2 <= rr - 9:
                        n2 = min(2, q2 - rr2)
                        emit_l2(rr2, n2)
                        rr2 += n2
                while rr2 < q2:
                    n2 = min(2, q2 - rr2)
                    emit_l2(rr2, n2)
                    rr2 += n2
                while pending:
                    pending.pop(0)()
                if last:
                    # deferred t5 row-H slot zeroing: strip k-1's deferred
                    # gather still reads the old slot content
                    nc.vector.memset(t5[:, q3 - a + 1 : q3 - a + 2, :], 0.0)

                # --- L3: rows [p3, q3), reads h2; mmA interleaved so t5
                # fills (and the gather can start) as h3 rows land ---
                mma_q = []
                rr = p3
                while rr < q3:
                    n = min(2, q3 - rr)
                    mma_q.append((rr, n))
                    rr += n
                ci4 = 0

                def emit_mma(rr, n, ci, a=a):
                    s = rr - a  # h3 slot
                    ps = pmain.tile([HID_C, n, W], f32, tag="ps")
                    nc.tensor.matmul(
                        ps, w4a_sb[:, :], h3[:, s : s + n, 1 : 1 + W],
                        start=True, stop=True,
                    )
                    d = rr - a + 1  # t5 slot
                    if (ci % 2 == 0 and not last) or (last and rr + n >= q3):
                        # last strip: the final chunk's copy gates the last
                        # gather piece -- DVE is idle there while ACT is
                        # backed up with the L3 drain
                        nc.vector.tensor_copy(t5[:, d : d + n, 1 : 1 + W], ps)
                    else:
                        nc.scalar.activation(
                            t5[:, d : d + n, 1 : 1 + W], ps, Ident)

                # last strip: 4-row gather pieces chase the mmA coverage
                # (each queues right behind its gating mmA copy), so the
                # drain tail is one small piece deep; the tap-sum runs in
                # two halves
                nr = b - a
                # gather piece list: piece (h0,h1r) reads t5 slots
                # [h0, h1r+2) <=> mmA rows <= a+h1r+1; the penultimate
                # piece is only 2 rows so its gate clears one mmA chunk
                # before the end and only a 2-row piece trails the drain
                qpieces = [(0, 4, True), (4, 8, False), (8, 12, False),
                           (12, 14, False)]
                qstate = [0]
                gstate = [0]

                def g_flush(cov, a=a, nr=nr):
                    # interior strips: half-1 gather can fire mid-L3 once
                    # the mmA coverage allows -- its descriptors clear the
                    # serialized queue long before the next strip's
                    # tap-sum pops need them
                    if not last and gstate[0] == 0 and cov >= a + nr // 2:
                        gather_piece(a, 0, nr // 2)
                        gstate[0] = 1

                def q_flush(cov, a=a, nr=nr):
                    while last and qstate[0] < len(qpieces):
                        h0, h1r, cen = qpieces[qstate[0]]
                        if cov < a + h1r + 1:
                            break
                        gather_piece(a, h0, h1r, fast=True, center=cen)
                        qstate[0] += 1
                        if qstate[0] == 2:
                            for e in l4b_half(a, a, a + nr // 2, 0,
                                              direct=True):
                                e()

                rr = p3
                ci3 = 0
                while rr < q3:
                    n = min(2, q3 - rr)
                    s = rr - a + 1  # h2 slot of center
                    ps = pmain.tile([HID_C, n, W], f32, tag="ps")
                    conv_chunk(ps, w3_sb, h2, s, n)
                    if ci3 % 2 == 0:
                        nc.scalar.activation(
                            h3[:, s - 1 : s - 1 + n, 1 : 1 + W], ps, Relu,
                            bias=b3_sb,
                        )
                    else:
                        nc.vector.tensor_scalar(
                            h3[:, s - 1 : s - 1 + n, 1 : 1 + W], ps, b3_sb,
                            0.0, Add, Max,
                        )
                    ci3 += 1
                    if mma_q and mma_q[0][0] + 2 < rr:
                        r0, n0 = mma_q.pop(0)
                        emit_mma(r0, n0, ci4)
                        ci4 += 1
                        q_flush(r0 + n0 - 1)
                        g_flush(r0 + n0 - 1)
                    rr += n
                if not last:
                    # drain mmA to 2 chunks, pre-roll the next strip's
                    # first L1 chunks (ready PE work covering the last h3
                    # copies' latency), then finish the drain
                    while len(mma_q) > 2:
                        r0, n0 = mma_q.pop(0)
                        emit_mma(r0, n0, ci4)
                        ci4 += 1
                    p1_n = b + 3
                    for j in range(PRE):
                        l1_chunk(
                            p1_n + 2 * j, 2, j, a=b, xb=xbufs[(k + 1) % 2])
                    while mma_q:
                        r0, n0 = mma_q.pop(0)
                        emit_mma(r0, n0, ci4)
                        ci4 += 1

                    # two half-strip gathers: the first fires before the
                    # last tap-partial copies land, unblocking the first
                    # sum-matmuls earlier
                    g_flush(H)
                    gather_piece(a, nr // 2, nr)
                    pending = (
                        l4b_half(a, a, a + nr // 2, 0)
                        + l4b_half(a, a + nr // 2, b, 1)
                    )
                else:
                    # final strip: finish the mmA drain (gather pieces and
                    # the half-1 tap-sum interleave via q_flush), then the
                    # final piece and the second half
                    while mma_q:
                        r0, n0 = mma_q.pop(0)
                        emit_mma(r0, n0, ci4)
                        ci4 += 1
                        q_flush(r0 + n0 - 1)
                    q_flush(H)
                    gather_piece(a, nr - 2, nr, fast=True, center=False,
                                 pool=True)
                    for e in l4b_half(a, a + nr // 2, b, 1, direct=True):
                        e()

            # flush (non-last-path safety; empty when loop ended on last)
            while pending:
                pending.pop(0)()

    nc.finalize()
    return nc


_NC_CACHE = {}


def _pack_inputs(x, w1, b1, w2, b2, w3, b3, w4, b4):
    x = np.asarray(x, dtype=np.float32)
    # 5 pre-shifted zero-padded copies of x, tap order
    # (center, up, down, left, right) matching the w1p packing
    x5 = np.zeros((B, 5, IN_C, H, W), np.float32)
    x5[:, 0] = x
    x5[:, 1, :, 1:, :] = x[:, :, :-1, :]   # up tap: row r reads x[r-1]
    x5[:, 2, :, :-1, :] = x[:, :, 1:, :]   # down
    x5[:, 3, :, :, 1:] = x[:, :, :, :-1]   # left
    x5[:, 4, :, :, :-1] = x[:, :, :, 1:]   # right
    x5 = x5.reshape(B, 5 * IN_C, H, W)
    # ones channel: K=31 L1 matmul row 30 carries b1, so the PSUM result
    # already includes the bias
    x5 = np.ascontiguousarray(
        np.concatenate([x5, np.ones((B, 1, H, W), np.float32)], axis=1))
    w1 = np.asarray(w1, dtype=np.float32)
    w2 = np.asarray(w2, dtype=np.float32)
    w3 = np.asarray(w3, dtype=np.float32)
    w4 = np.asarray(w4, dtype=np.float32)
    # w4a slabs: up@0-5, center@6-11, down@32-37, left@64-69, right@96-101
    w4a = np.zeros((HID_C, HID_C), np.float32)
    w4a[:, 0:OUT_C] = w4[:, :, 1].T          # up
    w4a[:, 6 : 6 + OUT_C] = w4[:, :, 0].T    # center
    w4a[:, 32 : 32 + OUT_C] = w4[:, :, 2].T  # down
    w4a[:, 64 : 64 + OUT_C] = w4[:, :, 3].T  # left
    w4a[:, 96 : 96 + OUT_C] = w4[:, :, 4].T  # right
    s6 = np.zeros((HID_C, OUT_C), np.float32)
    for base in (0, 6, 32, 64, 96):
        s6[base + np.arange(OUT_C), np.arange(OUT_C)] = 1.0
    s6b = np.array(s6)
    s6b[6:12, :] = 0.0
    s6 = s6.astype(ml_dtypes.bfloat16)
    s6b = s6b.astype(ml_dtypes.bfloat16)
    w1p = np.concatenate([
        w1.transpose(2, 1, 0).reshape(5 * IN_C, HID_C),
        np.asarray(b1, np.float32)[None, :],
    ])
    xw0 = np.zeros((B, 5 * IN_C + 1, 5, W), np.float32)
    xw0[:, :, 0, 0:HID_C] = w1p[None]
    xw0[:, :, 1:5, :] = x5[:, :, 0:4, :]
    common = {
        # w2p[ic, t, oc] = w2[oc, ic, t]
        "w2p": np.ascontiguousarray(w2.transpose(1, 2, 0)),
        "w3p": np.ascontiguousarray(w3.transpose(1, 2, 0)),
        "w4a": w4a,
        "s6": s6,
        "s6b": s6b,
        "w4c": np.ascontiguousarray(w4[:, :, 0].T),
        "b1": np.asarray(b1, np.float32),
        "b2": np.asarray(b2, np.float32),
        "b3": np.asarray(b3, np.float32),
        "b4": np.asarray(b4, np.float32),
    }
    return x5, xw0, common


def kernel(x, w1, b1, w2, b2, w3, b3, w4, b4):
    x5, xw0, common = _pack_inputs(x, w1, b1, w2, b2, w3, b3, w4, b4)
    if "nc" not in _NC_CACHE:
        _NC_CACHE["nc"] = _build()
    nc = _NC_CACHE["nc"]
    in_maps = [
        dict(common, x5p=x5[i], xw0=xw0[i]) for i in range(N_CORES)
    ]
    res = bass_utils.run_bass_kernel_spmd(nc, in_maps, core_ids=list(range(N_CORES)))
    out = np.stack([res.results[i]["y"] for i in range(N_CORES)], axis=0)
    return out

